# revision 7
# baseline (speedup 1.0000x reference)
"""Trainium2 Bass kernel for ArcShapeRadiusConfigVisibleNeighDist.

For each pedestrian i (N=8192):
  heading u_i = normalize(pos_i - past_i)
  over all j: dist_ij = |pos_j - pos_i|, visible iff angle(pos_j-pos_i, u_i)
  in [-35deg, 35deg) and j != i. Output = affine(clip(mean visible dist)).

Key reformulation (no atan2 anywhere):
  visible  <=>  rel . u_i > cos(35deg) * dist  <=>  dot/c > dist
  sq and dot/c are K-small matmuls on the TensorEngine with bf16 hi/lo
  split features (K is free on the PE), giving ~fp32 accuracy at bf16 speed.
Per 128-query x 1024-j tile:
  PE:  G1 = sq (+eps), G2 = dot/c           (4 matmuls N=512)
  ACT: dist = sqrt(G1) -> bf16
  DVE: ttr: mask = (G2 > dist), accum -> cnt
       stt: sd = max(dist,0) * mask, accum -> s
Epilogue: r = clip(s/max(cnt,1) * k + b, 0.5, 4.0); select by indexes.

Sharding: core k owns queries [k*1024, (k+1)*1024), full j set.
"""

import os
import numpy as np
import ml_dtypes

import concourse.bass as bass
import concourse.bacc as bacc
import concourse.mybir as mybir
import concourse.tile as tile
from contextlib import ExitStack
from concourse.alu_op_type import AluOpType
from concourse.bass_utils import run_bass_kernel_spmd

N = 8192
NCORES = 8
Q = N // NCORES            # 1024 queries per core
ITILES = Q // 128          # 8 partition tiles of queries
JCHUNK = 1024
NJC = N // JCHUNK          # 8 j-chunks
EPS = 0.05                 # sq guard: keeps diag excluded and sqrt input >= 0
COS_HALF = float(np.cos(70.0 * np.pi / 180.0 / 2.0))
MIN_R, MAX_R = 0.5, 4.0
MIN_D, MAX_D = 0.2, 5.0
SLOPE = (MAX_R - MIN_R) / (MAX_D - MIN_D)
OFFS = MIN_R - MIN_D * SLOPE

F32 = mybir.dt.float32
BF16 = mybir.dt.bfloat16
ACTF = mybir.ActivationFunctionType

_BF = ml_dtypes.bfloat16


def _split(x):
    """Split f64 array into bf16 hi + bf16 lo (returned as f64 of exact bf16 values)."""
    h = x.astype(_BF).astype(np.float64)
    l = (x - h).astype(_BF).astype(np.float64)
    return h, l


def _build_graph():
    nc = bacc.Bacc("TRN2", target_bir_lowering=False, debug=False,
                   num_devices=NCORES)
    qf1_d = nc.dram_tensor("qf1", [10, Q], BF16, kind="ExternalInput")
    qf2_d = nc.dram_tensor("qf2", [8, Q], BF16, kind="ExternalInput")
    jf_d = nc.dram_tensor("jf", [10, N], BF16, kind="ExternalInput")
    sel_d = nc.dram_tensor("sel", [128, 2 * ITILES], F32, kind="ExternalInput")
    out_d = nc.dram_tensor("out", [Q], F32, kind="ExternalOutput")

    with tile.TileContext(nc) as tc, ExitStack() as ctx:
        singles = ctx.enter_context(tc.tile_pool(name="singles", bufs=1))
        psum = ctx.enter_context(tc.tile_pool(name="psum", bufs=2, space="PSUM"))
        work = ctx.enter_context(tc.tile_pool(name="work", bufs=4))
        parts = ctx.enter_context(tc.tile_pool(name="parts", bufs=2))

        qf1 = singles.tile([10, Q], BF16)
        nc.sync.dma_start(qf1[:], qf1_d[:])
        qf2 = singles.tile([8, Q], BF16)
        nc.sync.dma_start(qf2[:], qf2_d[:])
        jf = singles.tile([10, N], BF16)
        nc.sync.dma_start(jf[:], jf_d[:])
        sel = singles.tile([128, 2 * ITILES], F32)
        nc.sync.dma_start(sel[:], sel_d[:])

        S = singles.tile([128, ITILES], F32)
        C = singles.tile([128, ITILES], F32)

        for it in range(ITILES):
            lhs1 = qf1[:, bass.ts(it, 128)]
            lhs2 = qf2[:, bass.ts(it, 128)]
            s_parts = parts.tile([128, NJC], F32, tag="sparts")
            c_parts = parts.tile([128, NJC], F32, tag="cparts")
            for jc in range(NJC):
                g1 = psum.tile([128, JCHUNK], F32, tag="g1")
                g2 = psum.tile([128, JCHUNK], F32, tag="g2")
                for h in range(JCHUNK // 512):
                    col = jc * JCHUNK + h * 512
                    nc.tensor.matmul(g1[:, h * 512:(h + 1) * 512], lhs1,
                                     jf[0:10, col:col + 512])
                for h in range(JCHUNK // 512):
                    col = jc * JCHUNK + h * 512
                    nc.tensor.matmul(g2[:, h * 512:(h + 1) * 512], lhs2,
                                     jf[0:8, col:col + 512])
                dist = work.tile([128, JCHUNK], BF16, tag="dist")
                nc.scalar.activation(dist[:], g1[:], ACTF.Sqrt)
                mask = work.tile([128, JCHUNK], BF16, tag="mask")
                nc.vector.scalar_tensor_tensor(
                    out=mask[:], in0=g2[:], scalar=0.0, in1=dist[:],
                    op0=AluOpType.bypass, op1=AluOpType.is_gt,
                    accum_out=c_parts[:, jc:jc + 1])
                sd = work.tile([128, JCHUNK], BF16, tag="sd")
                nc.vector.scalar_tensor_tensor(
                    out=sd[:], in0=dist[:], scalar=0.0, in1=mask[:],
                    op0=AluOpType.max, op1=AluOpType.mult,
                    accum_out=s_parts[:, jc:jc + 1])
            nc.vector.tensor_reduce(out=C[:, it:it + 1], in_=c_parts[:],
                                    axis=mybir.AxisListType.X, op=AluOpType.add)
            nc.vector.tensor_reduce(out=S[:, it:it + 1], in_=s_parts[:],
                                    axis=mybir.AxisListType.X, op=AluOpType.add)

        # epilogue on [128, ITILES]
        ep = singles.tile([128, 5 * ITILES], F32)
        cm = ep[:, 0:ITILES]
        rc = ep[:, ITILES:2 * ITILES]
        mean = ep[:, 2 * ITILES:3 * ITILES]
        t1 = ep[:, 3 * ITILES:4 * ITILES]
        res = ep[:, 4 * ITILES:5 * ITILES]
        nc.vector.tensor_scalar(out=cm, in0=C[:], scalar1=1.0, scalar2=None,
                                op0=AluOpType.max)
        nc.vector.reciprocal(out=rc, in_=cm)
        nc.vector.tensor_tensor(out=mean, in0=S[:], in1=rc, op=AluOpType.mult)
        nc.vector.tensor_scalar(out=t1, in0=mean, scalar1=float(SLOPE),
                                scalar2=float(OFFS), op0=AluOpType.mult,
                                op1=AluOpType.add)
        nc.vector.tensor_scalar(out=res, in0=t1, scalar1=float(MIN_R),
                                scalar2=float(MAX_R), op0=AluOpType.max,
                                op1=AluOpType.min)
        # select: out = radii + idxf * (res - radii)
        idxf = sel[:, 0:ITILES]
        radii = sel[:, ITILES:2 * ITILES]
        d1 = ep[:, 0:ITILES]       # reuse
        nc.vector.tensor_tensor(out=d1, in0=res, in1=radii, op=AluOpType.subtract)
        d2 = ep[:, ITILES:2 * ITILES]
        nc.vector.tensor_tensor(out=d2, in0=d1, in1=idxf, op=AluOpType.mult)
        fin = ep[:, 2 * ITILES:3 * ITILES]
        nc.vector.tensor_tensor(out=fin, in0=d2, in1=radii, op=AluOpType.add)

        out_view = out_d.ap().rearrange("(p i) -> p i", p=128)
        nc.sync.dma_start(out_view, fin)

    nc.compile()
    return nc


_CACHED_NC = None


def _get_nc():
    global _CACHED_NC
    if _CACHED_NC is None:
        _CACHED_NC = _build_graph()
    return _CACHED_NC


def _prep_inputs(past_ped_positions, ped_positions, indexes, all_radii):
    pos = np.asarray(ped_positions, np.float64)
    past = np.asarray(past_ped_positions, np.float64)
    v = pos - past
    vn = np.hypot(v[:, 0], v[:, 1])
    safe = np.where(vn > 0, vn, 1.0)
    ux = np.where(vn > 0, v[:, 0] / safe, 1.0)
    uy = np.where(vn > 0, v[:, 1] / safe, 0.0)

    px, py = pos[:, 0], pos[:, 1]
    nsq = px * px + py * py
    px_h, px_l = _split(px)
    py_h, py_l = _split(py)
    nsq_h, nsq_l = _split(nsq)
    ones = np.ones(N)
    jf = np.stack([px_h, px_l, px_h, py_h, py_l, py_h, ones, ones,
                   nsq_h, nsq_l]).astype(_BF)

    a = ux / COS_HALF
    b = uy / COS_HALF
    w = (ux * px + uy * py) / COS_HALF
    a_h, a_l = _split(a)
    b_h, b_l = _split(b)
    w_h, w_l = _split(w)
    nq_h, nq_l = _split(nsq + EPS)
    qf1_full = np.stack([-2 * px_h, -2 * px_h, -2 * px_l,
                         -2 * py_h, -2 * py_h, -2 * py_l,
                         nq_h, nq_l, ones, ones])  # [10, N]
    qf2_full = np.stack([a_h, a_h, a_l, b_h, b_h, b_l, -w_h, -w_l])  # [8, N]

    # column c of per-core qf holds local query (c % 128) * ITILES + c // 128
    cidx = np.arange(Q)
    perm = (cidx % 128) * ITILES + cidx // 128

    idxf = np.asarray(indexes).astype(np.float64)
    radii = np.asarray(all_radii, np.float64)

    in_maps = []
    for k in range(NCORES):
        sl = slice(k * Q, (k + 1) * Q)
        qf1_core = qf1_full[:, sl][:, perm].astype(_BF)
        qf2_core = qf2_full[:, sl][:, perm].astype(_BF)
        # sel[p, it] corresponds to local query p * ITILES + it
        sel = np.concatenate([
            idxf[sl].reshape(128, ITILES),
            radii[sl].reshape(128, ITILES)], axis=1).astype(np.float32)
        in_maps.append({"qf1": qf1_core, "qf2": qf2_core, "jf": jf,
                        "sel": sel})
    return in_maps


def kernel(past_ped_positions, ped_positions, indexes, all_radii,
           _trace=False, _trace_kwargs=None):
    nc = _get_nc()
    in_maps = _prep_inputs(past_ped_positions, ped_positions, indexes,
                           all_radii)
    kw = {}
    if _trace:
        kw = {"trace": True}
        if _trace_kwargs:
            kw.update(_trace_kwargs)
    res = run_bass_kernel_spmd(nc, in_maps, list(range(NCORES)), **kw)
    out = np.concatenate([np.asarray(res.results[k]["out"], np.float32)
                          for k in range(NCORES)])
    if _trace:
        kernel.last_results = res
    return out
